# revision 5
# baseline (speedup 1.0000x reference)
"""Trainium2 Bass kernel for nn_MixtureOfExpertsModel (moe_routing).

Computes, for x [65536, 1024] and 10 experts with 15 outputs each:
    miu = x @ expert_w^T + expert_b      (per expert)
    xi  = x @ gate_w^T + gate_b          (per expert)
    out = sum_e softmax_e(xi) * miu      -> [65536, 15]

Strategy: pure data parallel over 8 NeuronCores (8192 rows each); at fp16
the kernel is PE-bound (2.52 GMAC/core -> 153.6k PE cycles = 64 us), so the
design keeps the PE streaming 300-column fp16 matmuls back-to-back at the
~127 ns issue floor and pushes everything else off the critical path:

 * x is repacked on the host into per-slab blocks (512 rows; one contiguous
   8KB run per partition) so every HWDGE load is 128 large descriptors --
   the SDMA per-descriptor fixed cost (~130ns/engine) makes smaller
   descriptors drain far below line rate.
 * head: the weights ride the Sync ring while slab 0 rides the Scalar ring
   concurrently; PE warmup matmuls hold the HAM clock gate open until both
   land (~12.5 us; the framework preamble alone is ~6.6 us).
 * no bias matmul: the psum->SBUF eviction is a Vector tensor_add per
   HALF-slab (2 PSUM banks) that adds the host-replicated bias and narrows
   to fp16; psum pool = 4 x 2-bank tiles so the write-after-read handoff to
   the PE is fine-grained.  (tensor_reduce has no DVE 2x mode and the psum
   operand is fp32, so Vector runs ~3.7us/slab vs PE 4.08 -- fits.)
 * post-processing batched per 4-subtile slab: exp (Scalar) -> product
   (Vector, fp16 2x) -> segmented reduce over experts (Vector) -> per-slab
   reciprocal (Vector) -> final num*rden on the otherwise idle GpSimd.
 * rows are permuted (within each 2048-row group, partition p owns rows
   p*16..p*16+15) so each output store is 128 x 960B descriptors instead of
   2048 x 60B, removing the ~5 us exposed DMA tail.
"""

import sys

if "/opt/trn_rl_repo" not in sys.path:
    sys.path.insert(0, "/opt/trn_rl_repo")

import numpy as np

import concourse.bass as bass
import concourse.bacc as bacc
import concourse.tile as tile
import concourse.mybir as mybir
from concourse.bass_utils import run_bass_kernel_spmd

F32 = mybir.dt.float32
FP16 = mybir.dt.float16
BF16 = mybir.dt.bfloat16

MDT = FP16
NPDT = np.float16

BS = 65536
K = 1024
E = 10
O = 15
NCOL = 2 * E * O          # 300: cols 0..149 = expert (n=o*E+e), 150..299 = gate
NCORES = 8
RPC = BS // NCORES        # rows per core: 8192
KC = K // 128             # 8 contraction chunks
SLAB = 512                # rows per slab = 4 matmul subtiles
NSUB = SLAB // 128        # 4 subtiles per slab
NSLAB = RPC // SLAB       # 16 slabs per core
GROUP = 4                 # slabs per output group (2048 rows per out DMA)
NGRP = NSLAB // GROUP
PREFETCH = 4              # x slabs in flight ahead of compute
N_WARMUP = 22             # PE warmup matmuls (HAM clock-gate release)


def _build():
    nc = bacc.Bacc("TRN2", target_bir_lowering=False, debug=False,
                   num_devices=NCORES)
    # xt row k*128+q holds slab k's contiguous (j, c, m) block: j = subtile,
    # c = k-chunk, m = moving-row index p; q = k-chunk partition.
    xt = nc.dram_tensor("xt", [NSLAB * 128, NSUB * KC * 128], MDT,
                        kind="ExternalInput").ap()
    wt = nc.dram_tensor("wt", [128, KC * NCOL], MDT, kind="ExternalInput").ap()
    biasb = nc.dram_tensor("biasb", [128, NSUB * NCOL], F32,
                           kind="ExternalInput").ap()
    out = nc.dram_tensor("out", [RPC, O], F32, kind="ExternalOutput").ap()

    with tile.TileContext(nc) as tc:
        with (
            tc.tile_pool(name="const", bufs=1) as cp,
            tc.tile_pool(name="x", bufs=PREFETCH + 2) as xp,
            tc.tile_pool(name="ps", bufs=4, space="PSUM") as ps_pool,
            tc.tile_pool(name="mx", bufs=2) as mx_pool,
            tc.tile_pool(name="pe", bufs=2) as pe_pool,
            tc.tile_pool(name="nd", bufs=2) as nd_pool,
            tc.tile_pool(name="ob", bufs=2) as ob_pool,
        ):
            # The weights and slab 0 gate the first real matmul group: wt on
            # the Sync ring, slab 0 concurrently on the Scalar ring.
            wt_t = cp.tile([128, KC * NCOL], MDT, name="wt_t")
            nc.sync.dma_start(wt_t[:], wt[:])
            wt_v = wt_t[:].rearrange("p (c n) -> p c n", c=KC)
            xts = {}
            xt0 = xp.tile([128, NSUB * KC * 128], MDT, tag="xt", name="xt_0")
            nc.scalar.dma_start(xt0[:], xt[0:128, :])
            xts[0] = xt0
            biasb_t = cp.tile([128, NSUB, NCOL], F32, name="biasb_t")
            nc.scalar.dma_start(
                biasb_t[:], biasb[:].rearrange("p (s n) -> p s n", s=NSUB))

            # Warm up the PE's HAM clock gate while the weights and slab 0
            # stream in: matmuls on a memset tile, no DMA deps.  The warmup
            # psum tile takes one rotation slot of the psum pool; its banks
            # are recycled later (same-engine WAW, no readers).
            wu_in = cp.tile([128, NCOL], BF16, name="wu_in")
            nc.gpsimd.memset(wu_in[:], 0.125)
            wu_ps = ps_pool.tile([128, 2 * 512], F32, tag="ps", name="wu_ps")
            for _ in range(N_WARMUP):
                nc.tensor.matmul(
                    wu_ps[:, 0:NCOL], wu_in[:, 0:128], wu_in[:],
                    start=True, stop=True, skip_group_check=True,
                )

            # Prime the x pipeline (slabs 1..PREFETCH on the Sync ring).
            for k in range(1, min(1 + PREFETCH, NSLAB)):
                xt_t = xp.tile([128, NSUB * KC * 128], MDT, tag="xt",
                               name=f"xt_{k}")
                nc.sync.dma_start(xt_t[:], xt[k * 128:(k + 1) * 128, :])
                xts[k] = xt_t

            ob = None
            ndb = None
            for k in range(NSLAB):
                kin = k % GROUP
                if kin == 0:
                    ob = ob_pool.tile([128, GROUP * NSUB, O], F32, tag="ob",
                                      name=f"ob_{k // GROUP}")
                    ndb = nd_pool.tile([128, GROUP * NSUB, 2, O], F32,
                                       tag="ndb", name=f"ndb_{k // GROUP}")
                kp = k + 1 + PREFETCH
                if 0 < kp < NSLAB:
                    xt_t = xp.tile([128, NSUB * KC * 128], MDT, tag="xt",
                                   name=f"xt_{kp}")
                    nc.sync.dma_start(xt_t[:], xt[kp * 128:(kp + 1) * 128, :])
                    xts[kp] = xt_t
                xt_v = xts[k][:].rearrange(
                    "p (j c m) -> p j c m", j=NSUB, c=KC)

                mx = mx_pool.tile([128, NSUB, NCOL], MDT, tag="mx",
                                  name=f"mx_{k}")
                # Two half-slab psum tiles (2 banks each): the Vector evict
                # (+ bias add, narrow to fp16) releases PSUM to the PE at
                # half-slab granularity and is the sole PSUM reader.
                for h in range(2):
                    psum = ps_pool.tile([128, 2 * 512], F32, tag="ps",
                                        name=f"ps_{k}_{h}")
                    for jj in range(2):
                        j = 2 * h + jj
                        for c in range(KC):
                            nc.tensor.matmul(
                                psum[:, jj * 512:jj * 512 + NCOL],
                                xt_v[:, j, c, :],
                                wt_v[:, c, :],
                                start=(c == 0), stop=(c == KC - 1),
                            )
                    nc.vector.tensor_add(
                        mx[:, 2 * h:2 * h + 2, :],
                        psum[:].rearrange("p (s b) -> p s b", s=2)[:, :, 0:NCOL],
                        biasb_t[:, 2 * h:2 * h + 2, :],
                    )
                # pe[:, :, 1, :] = exp(xi); pe[:, :, 0, :] = exp(xi) * miu
                pe = pe_pool.tile([128, NSUB, 2, E * O], MDT, tag="pe",
                                  name=f"pe_{k}")
                nc.scalar.activation(
                    pe[:, :, 1, :], mx[:, :, E * O:NCOL],
                    mybir.ActivationFunctionType.Exp,
                )
                nc.vector.tensor_mul(
                    pe[:, :, 0, :], mx[:, :, 0:E * O], pe[:, :, 1, :])
                # Segmented sum over experts (e contiguous, n = o*E + e):
                # ndb[:, s, 0, o] = num, ndb[:, s, 1, o] = den.
                nc.vector.reduce_sum(
                    ndb[:, kin * NSUB:(kin + 1) * NSUB, :, :]
                    .rearrange("p s h o -> p (s h) o"),
                    pe[:].rearrange("p s h (o e) -> p (s h) o e", o=O, e=E),
                    axis=mybir.AxisListType.X,
                )
                rden = nd_pool.tile([128, NSUB, O], F32, tag="rden",
                                    name=f"rden_{k}")
                nc.vector.reciprocal_approx_fast(
                    rden[:], ndb[:, kin * NSUB:(kin + 1) * NSUB, 1, :])
                nc.vector.tensor_mul(
                    ob[:, kin * NSUB:(kin + 1) * NSUB, :],
                    ndb[:, kin * NSUB:(kin + 1) * NSUB, 0, :], rden[:])
                if kin == GROUP - 1:
                    g = k // GROUP
                    g0 = g * GROUP * SLAB
                    # rows r = g0 + p*16 + s (host permutes x to match)
                    nc.scalar.dma_start(
                        out[g0:g0 + GROUP * SLAB, :]
                        .rearrange("(p s) o -> p s o", p=128),
                        ob[:],
                    )
    nc.compile()
    return nc


_NC = None


def _get_nc():
    global _NC
    if _NC is None:
        _NC = _build()
    return _NC


def _prep_inputs(x, expert_w, expert_b, gate_w, gate_b):
    # o-major expert columns (n = o*E + e) so the on-chip segmented reduce
    # over experts reads contiguous runs.
    w = np.concatenate([
        np.asarray(expert_w, np.float32).reshape(E, O, K)
        .transpose(1, 0, 2).reshape(E * O, K),
        np.asarray(gate_w, np.float32).reshape(E, O, K)
        .transpose(1, 0, 2).reshape(E * O, K),
    ], axis=0)                                   # [300, K], col n = o*E + e
    b = np.concatenate([
        np.asarray(expert_b, np.float32).reshape(E, O).T.reshape(E * O),
        np.asarray(gate_b, np.float32).reshape(E, O).T.reshape(E * O),
    ]).reshape(1, 1, NCOL)
    # wt[q, (c, n)] = w[n, c*128+q]
    wt = np.ascontiguousarray(
        w.reshape(NCOL, KC, 128).transpose(2, 1, 0).astype(NPDT)
        .reshape(128, KC * NCOL))
    biasb = np.ascontiguousarray(
        np.broadcast_to(b, (128, NSUB, NCOL)).reshape(128, NSUB * NCOL)
        .astype(np.float32))
    # Row permutation: within each 2048-row group g of a core, partition p
    # owns rows g*2048 + p*16 + kin*4 + j (slab k = g*4+kin, subtile j).
    # Moving-row index m = p; block layout per slab-row q is (j, c, m).
    x16 = np.asarray(x).astype(NPDT)
    arr = x16.reshape(NCORES, NGRP, 128, GROUP, NSUB, KC, 128)
    #                 core    g     p    kin    j    c   q
    xt = np.ascontiguousarray(arr.transpose(0, 1, 3, 6, 4, 5, 2)) \
        .reshape(NCORES, NSLAB * 128, NSUB * KC * 128)
    in_maps = [{"xt": xt[i], "wt": wt, "biasb": biasb}
               for i in range(NCORES)]
    return in_maps


def _run(in_maps, **kw):
    res = run_bass_kernel_spmd(
        _get_nc(), in_maps, core_ids=list(range(NCORES)), **kw)
    out = np.concatenate([r["out"] for r in res.results], axis=0)
    return out, res


def kernel(x, expert_w, expert_b, gate_w, gate_b):
    in_maps = _prep_inputs(x, expert_w, expert_b, gate_w, gate_b)
    out, _ = _run(in_maps)
    return out


def kernel_traced(x, expert_w, expert_b, gate_w, gate_b, **kw):
    """Like kernel() but returns (out, BassKernelResults) with an NTFF trace."""
    in_maps = _prep_inputs(x, expert_w, expert_b, gate_w, gate_b)
    return _run(in_maps, trace=True, **kw)
